# revision 2
# baseline (speedup 1.0000x reference)
"""GCN inference kernel v2 (y = D^-1/2 A D^-1/2 (x @ W.T)) on 8 NeuronCores.

Changes vs v1 (all aimed at the measured bottlenecks):
  - Table rows are bf16 DUPLICATED [h,h] (256B): the SWDGE gather lands
    matmul-ready bf16 (no on-chip cast), rhs = g[:, sl, 0:64].
  - One-hot B matrices are HOST-precomputed fp8 (exact 0/1) and streamed
    from DRAM: the DVE does nothing in phase B, removing the POOL-port
    contention that slowed SWDGE descriptor generation (3.3 -> ~1.9 ns/idx),
    and fp8 lhsT enables fast weight load (PE ~40ns/slice vs 428ns fp32).
  - Phase A is replicated per core (no AllGather): x is pre-scaled by
    dinv[src] host-side, shipped bf16; each core computes the full table
    bucket-by-bucket so phase-B gathers for bucket 0 start early.
"""

import math
from dataclasses import dataclass, field

import numpy as np
import ml_dtypes

import concourse.bacc as bacc
import concourse.bass as bass
import concourse.mybir as mybir
import concourse.tile as tile
from concourse.bass_utils import run_bass_kernel_spmd

P = 128
FIN = 128
FOUT = 64

f32 = mybir.dt.float32
bf16 = mybir.dt.bfloat16
fp8 = mybir.dt.float8e4
i16 = mybir.dt.int16


@dataclass
class Prm:
    N: int = 100000
    C: int = 8
    WG: int = 512  # nodes per phase-A write group
    BKCAP: int = 25600  # table rows per gather bucket (<= 32767)
    SWD: int = 512  # dst nodes per superwindow
    S_CAP: int = 48  # max slices per dma_gather call
    J: int = field(init=False)
    NS: int = field(init=False)
    N2: int = field(init=False)
    NG: int = field(init=False)
    NBK: int = field(init=False)
    TPSW: int = field(init=False)
    NSW: int = field(init=False)

    def __post_init__(self):
        assert self.WG % P == 0 and self.BKCAP % self.WG == 0
        assert self.BKCAP <= 32767 and self.SWD % P == 0 and self.N % self.C == 0
        self.J = self.WG // P
        self.NS = self.N // self.C
        self.N2 = ((self.N + self.WG - 1) // self.WG) * self.WG
        self.NG = self.N2 // self.WG
        self.NBK = (self.N2 + self.BKCAP - 1) // self.BKCAP
        self.TPSW = self.SWD // P
        self.NSW = (self.NS + self.SWD - 1) // self.SWD


def _rmap(prm, n):
    """node id -> table row (group-wrapped so phase-A tiles write rows
    [g*WG + j*P + p] with partition p, matching the matmul output layout)."""
    return prm.WG * (n // prm.WG) + prm.J * (n % P) + (n % prm.WG) // P


def _wrap_idx(vals16):
    k = vals16.shape[0]
    w16 = vals16.reshape(k // 16, 16).T
    return np.tile(w16, (8, 1))


@dataclass
class CallMeta:
    sw: int
    bk: int
    k: int
    S: int
    icol: int  # column offset into gidx (8 * slice offset)
    bcol: int  # slice offset into the B array


def _schedule(prm, n_sl_u):
    calls = []
    mms_by_sw = []
    icol = 0
    bcol = 0
    for sw in range(prm.NSW):
        for bk in range(prm.NBK):
            nsl = sum(int(n_sl_u[sw][bk][t]) for t in range(prm.TPSW))
            for k, a in enumerate(range(0, nsl, prm.S_CAP)):
                S = min(prm.S_CAP, nsl - a)
                calls.append(CallMeta(sw, bk, k, S, icol, bcol))
                icol += 8 * S
                bcol += S
        mms = []
        seen = [0] * prm.TPSW
        tot = [
            sum(int(n_sl_u[sw][bk][t]) for bk in range(prm.NBK))
            for t in range(prm.TPSW)
        ]
        for bk in range(prm.NBK):
            s0 = 0
            for t in range(prm.TPSW):
                for _ in range(int(n_sl_u[sw][bk][t])):
                    mms.append((bk, s0, t, seen[t] == 0, seen[t] == tot[t] - 1))
                    seen[t] += 1
                    s0 += 1
        mms_by_sw.append(mms)
    return calls, mms_by_sw, icol, bcol


def _host_prep(x, edge_index, W, prm):
    N, C, NS = prm.N, prm.C, prm.NS
    src = np.asarray(edge_index[0]).astype(np.int64)
    dst = np.asarray(edge_index[1]).astype(np.int64)
    x = np.asarray(x, dtype=np.float32)
    W = np.asarray(W, dtype=np.float32)

    deg = np.bincount(dst, minlength=N).astype(np.float64)
    dinv = np.where(deg > 0, 1.0 / np.sqrt(np.maximum(deg, 1.0)), 0.0).astype(
        np.float32
    )

    r_of = _rmap(prm, np.arange(N, dtype=np.int64))
    bk_of = (r_of // prm.BKCAP).astype(np.int64)
    rel_of = (r_of % prm.BKCAP).astype(np.int64)

    core_e = dst // NS
    edl = dst - core_e * NS
    sw_e = edl // prm.SWD
    t_e = (edl % prm.SWD) // P
    q_e = edl % P
    bk_e = bk_of[src]
    rel_e = rel_of[src]

    ncell = prm.NSW * prm.NBK * prm.TPSW
    counts = np.zeros((C, ncell), dtype=np.int64)
    percore = []
    for c in range(C):
        m = core_e == c
        order = np.lexsort((edl[m], t_e[m], bk_e[m], sw_e[m]))
        cell = (sw_e[m] * prm.NBK + bk_e[m]) * prm.TPSW + t_e[m]
        counts[c] = np.bincount(cell, minlength=ncell)
        percore.append(
            {"rel": rel_e[m][order], "q": q_e[m][order], "cell": cell[order]}
        )

    n_sl_u = np.zeros((prm.NSW, prm.NBK, prm.TPSW), dtype=np.int64)
    cmax = counts.max(axis=0).reshape(prm.NSW, prm.NBK, prm.TPSW)
    n_sl_u[:] = (cmax + P - 1) // P
    for sw in range(prm.NSW):
        rows_sw = min(prm.SWD, NS - sw * prm.SWD)
        ntile = (rows_sw + P - 1) // P
        for t in range(ntile):
            if n_sl_u[sw, :, t].sum() == 0:
                n_sl_u[sw, 0, t] = 1

    calls, mms_by_sw, icols, bcols = _schedule(prm, n_sl_u)

    cell_sl = n_sl_u.reshape(ncell)
    cell_off = np.zeros(ncell, dtype=np.int64)
    np.cumsum(cell_sl[:-1], out=cell_off[1:])
    S_total = int(cell_sl.sum())

    ONE8 = np.float32(1.0).astype(ml_dtypes.float8_e4m3).view(np.uint8)
    gidx_all = np.zeros((C, P, icols), dtype=np.int16)
    bmat_all = np.zeros((C, P, bcols * P), dtype=np.uint8)
    for c in range(C):
        pc = percore[c]
        ne = pc["cell"].shape[0]
        cc = counts[c]
        starts = np.zeros(ncell, dtype=np.int64)
        np.cumsum(cc[:-1], out=starts[1:])
        rank = np.arange(ne, dtype=np.int64) - starts[pc["cell"]]
        pos = cell_off[pc["cell"]] * P + rank  # slot position in slice stream
        vals = np.zeros(S_total * P, dtype=np.int16)
        vals[pos] = pc["rel"].astype(np.int16)
        # B one-hot: slot (slice s, partition p) -> column q (dst lane)
        bm = np.zeros((S_total * P, P), dtype=np.uint8)
        bm[pos, pc["q"]] = ONE8
        bm = bm.reshape(S_total, P, P)  # [slice, edge-part, dstcol]
        # per-call packing (calls' slices are consecutive in the stream)
        for cm in calls:
            seg = vals[cm.bcol * P : (cm.bcol + cm.S) * P]
            gidx_all[c, :, cm.icol : cm.icol + 8 * cm.S] = _wrap_idx(seg)
        bmat_all[c] = (
            bm.transpose(1, 0, 2).reshape(P, S_total * P)
        )
    del bm

    # phase-A inputs: x pre-scaled by dinv, transposed, gather-row order
    xp = (x * dinv[:, None]).astype(np.float32)
    xTs = np.zeros((FIN, prm.N2), dtype=np.float32)
    # natural node order: the phase-A write rearrange maps stream column
    # (g, j, p) to table row WG*g + J*p + j == _rmap(node), matching gidx.
    xTs[:, : prm.N] = xp.T
    xT16 = xTs.astype(ml_dtypes.bfloat16)
    WT = np.ascontiguousarray(W.T).astype(ml_dtypes.bfloat16)

    dinvD = np.zeros((C, P, prm.NSW * prm.TPSW), dtype=np.float32)
    w_idx = np.arange(prm.NSW * prm.TPSW)
    for c in range(C):
        node = c * NS + w_idx[:, None] * P + np.arange(P)[None, :]
        ok = node < (c + 1) * NS
        dv = np.where(ok, dinv[np.minimum(node, N - 1)], 0.0)
        dinvD[c][np.arange(P)[None, :], w_idx[:, None]] = dv

    inputs = []
    for c in range(C):
        inputs.append(
            {
                "xT": xT16,
                "WT": WT,
                "dinvD": dinvD[c],
                "gidx": gidx_all[c],
                "bmat": bmat_all[c].view(ml_dtypes.float8_e4m3),
            }
        )
    return inputs, calls, mms_by_sw


def _split_sync_waits(nc):
    for bb in nc.main_func.blocks:
        insts = bb.instructions
        i = 0
        while i < len(insts):
            ins = insts[i]
            si = ins.sync_info
            if si is not None and si.on_wait is not None and len(si.on_wait) > 1:
                waits = list(si.on_wait)
                keep, extra = waits[-1:], waits[:-1]
                k = 0
                while extra:
                    chunk, extra = extra[:1], extra[1:]
                    nop = mybir.InstNoOp(name=f"{ins.name}-ws{k}", ins=[], outs=[])
                    nop.engine = ins.engine
                    nop.sync_info = mybir.SyncInfo(on_wait=chunk, on_update=[])
                    nc.register_instruction(nop)
                    insts.insert(i, nop)
                    i += 1
                    k += 1
                ins.sync_info = mybir.SyncInfo(
                    on_wait=keep, on_update=list(si.on_update or [])
                )
            i += 1


def _build_program(prm, calls, mms_by_sw, icols, bcols):
    nc = bacc.Bacc("TRN2", num_swdge_queues=4)

    xT = nc.declare_dram_parameter("xT", [FIN, prm.N2], bf16, isOutput=False)
    WT = nc.declare_dram_parameter("WT", [FIN, FOUT], bf16, isOutput=False)
    dinvD = nc.declare_dram_parameter(
        "dinvD", [P, prm.NSW * prm.TPSW], f32, isOutput=False
    )
    gidx = nc.declare_dram_parameter("gidx", [P, icols], i16, isOutput=False)
    bmat = nc.declare_dram_parameter("bmat", [P, bcols * P], fp8, isOutput=False)
    y = nc.declare_dram_parameter("y", [prm.NS, FOUT], f32, isOutput=True)
    TBLS = [
        nc.dram_tensor(f"tbl{b}", [prm.BKCAP, 2 * FOUT], bf16)
        for b in range(prm.NBK)
    ]
    GPB = prm.BKCAP // prm.WG  # phase-A groups per bucket

    with tile.TileContext(nc) as tc:
        with tc.tile_pool(name="const", bufs=1) as cpool:
            wt_sb = cpool.tile([FIN, FOUT], bf16, tag="wt")
            nc.sync.dma_start(out=wt_sb[:], in_=WT[:])
            dinvD_sb = cpool.tile([P, prm.NSW * prm.TPSW], f32, tag="dd")
            nc.sync.dma_start(out=dinvD_sb[:], in_=dinvD[:])

            # ------- Phase A (replicated): table rows, bucket order -------
            with (
                tc.tile_pool(name="pa", bufs=4) as pa,
                tc.tile_pool(name="psa", bufs=4, space="PSUM") as psa,
            ):
                for g in range(prm.NG):
                    xt = pa.tile([P, prm.WG], bf16, tag="xt")
                    nc.sync.dma_start(
                        out=xt[:], in_=xT[:, g * prm.WG : (g + 1) * prm.WG]
                    )
                    hps = psa.tile([P, prm.J * FOUT], f32, tag="hps")
                    for j in range(prm.J):
                        nc.tensor.matmul(
                            out=hps[:, j * FOUT : (j + 1) * FOUT],
                            lhsT=xt[:, j * P : (j + 1) * P],
                            rhs=wt_sb[:],
                            start=True,
                            stop=True,
                        )
                    tsb = pa.tile([P, prm.J, 2 * FOUT], bf16, tag="tsb")
                    nc.vector.tensor_copy(
                        tsb[:, :, 0:FOUT],
                        hps[:].rearrange("p (j f) -> p j f", f=FOUT),
                    )
                    nc.vector.tensor_copy(
                        tsb[:, :, FOUT : 2 * FOUT],
                        hps[:].rearrange("p (j f) -> p j f", f=FOUT),
                    )
                    base = prm.WG * (g % GPB)
                    nc.sync.dma_start(
                        out=TBLS[g // GPB][base : base + prm.WG, :].rearrange(
                            "(p j) f -> p j f", j=prm.J
                        ),
                        in_=tsb[:],
                    )

            # ------- Phase B: gather + one-hot matmuls -------
            qctr = [0]
            calls_by_sw = [[] for _ in range(prm.NSW)]
            for cm in calls:
                calls_by_sw[cm.sw].append(cm)
            S_MAX = max((cm.S for cm in calls), default=1)
            with (
                tc.tile_pool(name="pidx", bufs=6) as pidx,
                tc.tile_pool(name="pg", bufs=6) as pg,
                tc.tile_pool(name="pb", bufs=6) as pb,
                tc.tile_pool(name="py", bufs=2) as py,
                tc.tile_pool(name="psb", bufs=2, space="PSUM") as psb,
            ):
                for sw in range(prm.NSW):
                    if not calls_by_sw[sw]:
                        continue
                    tiles = {}
                    for cm in calls_by_sw[sw]:
                        S = cm.S
                        idx_t = pidx.tile([P, 8 * S_MAX], i16, tag="idx")
                        nc.scalar.dma_start(
                            out=idx_t[:, : 8 * S],
                            in_=gidx[:, cm.icol : cm.icol + 8 * S],
                        )
                        b_t = pb.tile([P, S_MAX, P], fp8, tag="b")
                        nc.sync.dma_start(
                            out=b_t[:, :S, :],
                            in_=bmat[:, cm.bcol * P : (cm.bcol + S) * P].rearrange(
                                "p (s q) -> p s q", q=P
                            ),
                        )
                        g_t = pg.tile([P, S_MAX, 2 * FOUT], bf16, tag="g")
                        nc.gpsimd.dma_gather(
                            out_ap=g_t[:, :S, :],
                            in_ap=TBLS[cm.bk][:],
                            idxs_ap=idx_t[:, : 8 * S],
                            num_idxs=S * P,
                            num_idxs_reg=S * P,
                            elem_size=2 * FOUT,
                            single_packet=False,
                            queue_num=qctr[0] % 4,
                        )
                        qctr[0] += 1
                        tiles[(cm.bk, cm.k)] = (g_t, b_t)
                    psum_t = [
                        psb.tile([P, FOUT], f32, tag=f"acc{t}", name=f"acc{t}")
                        for t in range(prm.TPSW)
                    ]
                    for bk, s, t, st, sp in mms_by_sw[sw]:
                        g_t, b_t = tiles[(bk, s // prm.S_CAP)]
                        sl = s % prm.S_CAP
                        nc.tensor.matmul(
                            out=psum_t[t][:],
                            lhsT=b_t[:, sl, :],
                            rhs=g_t[:, sl, 0:FOUT],
                            start=st,
                            stop=sp,
                        )
                    rows_sw = min(prm.SWD, prm.NS - sw * prm.SWD)
                    nt = (rows_sw + P - 1) // P
                    ysb = py.tile([P, prm.TPSW, FOUT], f32, tag="ysb")
                    for t in range(nt):
                        w = sw * prm.TPSW + t
                        nc.scalar.activation(
                            out=ysb[:, t, :],
                            in_=psum_t[t][:],
                            func=mybir.ActivationFunctionType.Copy,
                            scale=dinvD_sb[:, w : w + 1],
                        )
                    for t in range(nt):
                        rt = min(P, rows_sw - t * P)
                        r0 = sw * prm.SWD + t * P
                        nc.scalar.dma_start(
                            out=y[r0 : r0 + rt, :], in_=ysb[:rt, t, :]
                        )

    nc.compile()
    _split_sync_waits(nc)
    return nc


def _get_program_and_prep(x, edge_index, W, prm):
    inputs, calls, mms_by_sw = _host_prep(x, edge_index, W, prm)
    icols = sum(8 * cm.S for cm in calls)
    bcols = sum(cm.S for cm in calls)
    nc = _build_program(prm, calls, mms_by_sw, icols, bcols)
    return nc, inputs


def kernel(x, edge_index, W):
    prm = Prm(N=int(x.shape[0]))
    nc, inputs = _get_program_and_prep(x, edge_index, W, prm)
    res = run_bass_kernel_spmd(nc, inputs, list(range(prm.C)))
    y = np.concatenate([res.results[c]["y"] for c in range(prm.C)], axis=0)
    return y.astype(np.float32)


def run_with_trace(x, edge_index, W, trace_cores=None):
    prm = Prm(N=int(x.shape[0]))
    nc, inputs = _get_program_and_prep(x, edge_index, W, prm)
    res = run_bass_kernel_spmd(
        nc, inputs, list(range(prm.C)), trace=True, trace_cores=trace_cores
    )
    y = np.concatenate([res.results[c]["y"] for c in range(prm.C)], axis=0)
    return y.astype(np.float32), res


# revision 3
# speedup vs baseline: 1.2266x; 1.2266x over previous
"""GCN inference kernel v2 (y = D^-1/2 A D^-1/2 (x @ W.T)) on 8 NeuronCores.

Changes vs v1 (all aimed at the measured bottlenecks):
  - Table rows are bf16 DUPLICATED [h,h] (256B): the SWDGE gather lands
    matmul-ready bf16 (no on-chip cast), rhs = g[:, sl, 0:64].
  - One-hot B matrices are HOST-precomputed fp8 (exact 0/1) and streamed
    from DRAM: the DVE does nothing in phase B, removing the POOL-port
    contention that slowed SWDGE descriptor generation (3.3 -> ~1.9 ns/idx),
    and fp8 lhsT enables fast weight load (PE ~40ns/slice vs 428ns fp32).
  - Phase A is replicated per core (no AllGather): x is pre-scaled by
    dinv[src] host-side, shipped bf16; each core computes the full table
    bucket-by-bucket so phase-B gathers for bucket 0 start early.
"""

import math
from dataclasses import dataclass, field

import numpy as np
import ml_dtypes

import concourse.bacc as bacc
import concourse.bass as bass
import concourse.mybir as mybir
import concourse.tile as tile
from concourse.bass_utils import run_bass_kernel_spmd

P = 128
FIN = 128
FOUT = 64

f32 = mybir.dt.float32
bf16 = mybir.dt.bfloat16
fp8 = mybir.dt.float8e4
i16 = mybir.dt.int16


@dataclass
class Prm:
    N: int = 100000
    C: int = 8
    WG: int = 512  # nodes per phase-A write group
    BKCAP: int = 25600  # table rows per gather bucket (<= 32767)
    SWD: int = 512  # dst nodes per superwindow
    S_CAP: int = 48  # max slices per dma_gather call
    J: int = field(init=False)
    NS: int = field(init=False)
    N2: int = field(init=False)
    NG: int = field(init=False)
    NBK: int = field(init=False)
    TPSW: int = field(init=False)
    NSW: int = field(init=False)

    def __post_init__(self):
        assert self.WG % P == 0 and self.BKCAP % self.WG == 0
        assert self.BKCAP <= 32767 and self.SWD % P == 0 and self.N % self.C == 0
        self.J = self.WG // P
        self.NS = self.N // self.C
        self.N2 = ((self.N + self.WG - 1) // self.WG) * self.WG
        self.NG = self.N2 // self.WG
        self.NBK = (self.N2 + self.BKCAP - 1) // self.BKCAP
        self.TPSW = self.SWD // P
        self.NSW = (self.NS + self.SWD - 1) // self.SWD


def _rmap(prm, n):
    """node id -> table row (group-wrapped so phase-A tiles write rows
    [g*WG + j*P + p] with partition p, matching the matmul output layout)."""
    return prm.WG * (n // prm.WG) + prm.J * (n % P) + (n % prm.WG) // P


def _wrap_idx(vals16):
    k = vals16.shape[0]
    w16 = vals16.reshape(k // 16, 16).T
    return np.tile(w16, (8, 1))


@dataclass
class CallMeta:
    sw: int
    bk: int
    k: int
    S: int
    icol: int  # column offset into gidx (8 * slice offset)
    bcol: int  # slice offset into the B array


def _schedule(prm, n_sl_u):
    calls = []
    mms_by_sw = []
    icol = 0
    bcol = 0
    for sw in range(prm.NSW):
        for bk in range(prm.NBK):
            nsl = sum(int(n_sl_u[sw][bk][t]) for t in range(prm.TPSW))
            for k, a in enumerate(range(0, nsl, prm.S_CAP)):
                S = min(prm.S_CAP, nsl - a)
                calls.append(CallMeta(sw, bk, k, S, icol, bcol))
                icol += 8 * S
                bcol += S
        mms = []
        seen = [0] * prm.TPSW
        tot = [
            sum(int(n_sl_u[sw][bk][t]) for bk in range(prm.NBK))
            for t in range(prm.TPSW)
        ]
        for bk in range(prm.NBK):
            s0 = 0
            for t in range(prm.TPSW):
                for _ in range(int(n_sl_u[sw][bk][t])):
                    mms.append((bk, s0, t, seen[t] == 0, seen[t] == tot[t] - 1))
                    seen[t] += 1
                    s0 += 1
        mms_by_sw.append(mms)
    return calls, mms_by_sw, icol, bcol


def _host_prep(x, edge_index, W, prm):
    N, C, NS = prm.N, prm.C, prm.NS
    src = np.asarray(edge_index[0]).astype(np.int64)
    dst = np.asarray(edge_index[1]).astype(np.int64)
    x = np.asarray(x, dtype=np.float32)
    W = np.asarray(W, dtype=np.float32)

    deg = np.bincount(dst, minlength=N).astype(np.float64)
    dinv = np.where(deg > 0, 1.0 / np.sqrt(np.maximum(deg, 1.0)), 0.0).astype(
        np.float32
    )

    r_of = _rmap(prm, np.arange(N, dtype=np.int64))
    bk_of = (r_of // prm.BKCAP).astype(np.int64)
    rel_of = (r_of % prm.BKCAP).astype(np.int64)

    core_e = dst // NS
    edl = dst - core_e * NS
    sw_e = edl // prm.SWD
    t_e = (edl % prm.SWD) // P
    q_e = edl % P
    bk_e = bk_of[src]
    rel_e = rel_of[src]

    ncell = prm.NSW * prm.NBK * prm.TPSW
    counts = np.zeros((C, ncell), dtype=np.int64)
    percore = []
    for c in range(C):
        m = core_e == c
        order = np.lexsort((edl[m], t_e[m], bk_e[m], sw_e[m]))
        cell = (sw_e[m] * prm.NBK + bk_e[m]) * prm.TPSW + t_e[m]
        counts[c] = np.bincount(cell, minlength=ncell)
        percore.append(
            {"rel": rel_e[m][order], "q": q_e[m][order], "cell": cell[order]}
        )

    n_sl_u = np.zeros((prm.NSW, prm.NBK, prm.TPSW), dtype=np.int64)
    cmax = counts.max(axis=0).reshape(prm.NSW, prm.NBK, prm.TPSW)
    n_sl_u[:] = (cmax + P - 1) // P
    for sw in range(prm.NSW):
        rows_sw = min(prm.SWD, NS - sw * prm.SWD)
        ntile = (rows_sw + P - 1) // P
        for t in range(ntile):
            if n_sl_u[sw, :, t].sum() == 0:
                n_sl_u[sw, 0, t] = 1

    calls, mms_by_sw, icols, bcols = _schedule(prm, n_sl_u)

    cell_sl = n_sl_u.reshape(ncell)
    cell_off = np.zeros(ncell, dtype=np.int64)
    np.cumsum(cell_sl[:-1], out=cell_off[1:])
    S_total = int(cell_sl.sum())

    ONE8 = np.float32(1.0).astype(ml_dtypes.float8_e4m3).view(np.uint8)
    gidx_all = np.zeros((C, P, icols), dtype=np.int16)
    bmat_all = np.zeros((C, P, bcols * P), dtype=np.uint8)
    for c in range(C):
        pc = percore[c]
        ne = pc["cell"].shape[0]
        cc = counts[c]
        starts = np.zeros(ncell, dtype=np.int64)
        np.cumsum(cc[:-1], out=starts[1:])
        rank = np.arange(ne, dtype=np.int64) - starts[pc["cell"]]
        pos = cell_off[pc["cell"]] * P + rank  # slot position in slice stream
        vals = np.zeros(S_total * P, dtype=np.int16)
        vals[pos] = pc["rel"].astype(np.int16)
        # B one-hot: slot (slice s, partition p) -> column q (dst lane)
        bm = np.zeros((S_total * P, P), dtype=np.uint8)
        bm[pos, pc["q"]] = ONE8
        bm = bm.reshape(S_total, P, P)  # [slice, edge-part, dstcol]
        # per-call packing (calls' slices are consecutive in the stream)
        for cm in calls:
            seg = vals[cm.bcol * P : (cm.bcol + cm.S) * P]
            gidx_all[c, :, cm.icol : cm.icol + 8 * cm.S] = _wrap_idx(seg)
        bmat_all[c] = (
            bm.transpose(1, 0, 2).reshape(P, S_total * P)
        )
    del bm

    # phase-A inputs: x pre-scaled by dinv, transposed, gather-row order
    xp = (x * dinv[:, None]).astype(np.float32)
    xTs = np.zeros((FIN, prm.N2), dtype=np.float32)
    # natural node order: the phase-A write rearrange maps stream column
    # (g, j, p) to table row WG*g + J*p + j == _rmap(node), matching gidx.
    xTs[:, : prm.N] = xp.T
    xT16 = xTs.astype(ml_dtypes.bfloat16)
    WT = np.ascontiguousarray(W.T).astype(ml_dtypes.bfloat16)

    dinvD = np.zeros((C, P, prm.NSW * prm.TPSW), dtype=np.float32)
    w_idx = np.arange(prm.NSW * prm.TPSW)
    for c in range(C):
        node = c * NS + w_idx[:, None] * P + np.arange(P)[None, :]
        ok = node < (c + 1) * NS
        dv = np.where(ok, dinv[np.minimum(node, N - 1)], 0.0)
        dinvD[c][np.arange(P)[None, :], w_idx[:, None]] = dv

    inputs = []
    for c in range(C):
        inputs.append(
            {
                "xT": xT16,
                "WT": WT,
                "dinvD": dinvD[c],
                "gidx": gidx_all[c],
                "bmat": bmat_all[c].view(ml_dtypes.float8_e4m3),
            }
        )
    return inputs, calls, mms_by_sw


def _split_sync_waits(nc):
    for bb in nc.main_func.blocks:
        insts = bb.instructions
        i = 0
        while i < len(insts):
            ins = insts[i]
            si = ins.sync_info
            if si is not None and si.on_wait is not None and len(si.on_wait) > 1:
                waits = list(si.on_wait)
                keep, extra = waits[-1:], waits[:-1]
                k = 0
                while extra:
                    chunk, extra = extra[:1], extra[1:]
                    nop = mybir.InstNoOp(name=f"{ins.name}-ws{k}", ins=[], outs=[])
                    nop.engine = ins.engine
                    nop.sync_info = mybir.SyncInfo(on_wait=chunk, on_update=[])
                    nc.register_instruction(nop)
                    insts.insert(i, nop)
                    i += 1
                    k += 1
                ins.sync_info = mybir.SyncInfo(
                    on_wait=keep, on_update=list(si.on_update or [])
                )
            i += 1


def _build_program(prm, calls, mms_by_sw, icols, bcols):
    nc = bacc.Bacc("TRN2", num_swdge_queues=4)

    xT = nc.declare_dram_parameter("xT", [FIN, prm.N2], bf16, isOutput=False)
    WT = nc.declare_dram_parameter("WT", [FIN, FOUT], bf16, isOutput=False)
    dinvD = nc.declare_dram_parameter(
        "dinvD", [P, prm.NSW * prm.TPSW], f32, isOutput=False
    )
    gidx = nc.declare_dram_parameter("gidx", [P, icols], i16, isOutput=False)
    bmat = nc.declare_dram_parameter("bmat", [P, bcols * P], fp8, isOutput=False)
    y = nc.declare_dram_parameter("y", [prm.NS, FOUT], f32, isOutput=True)
    TBL = nc.dram_tensor("tbl", [prm.N2, 2 * FOUT], bf16)

    with tile.TileContext(nc) as tc:
        with tc.tile_pool(name="const", bufs=1) as cpool:
            wt_sb = cpool.tile([FIN, FOUT], bf16, tag="wt")
            nc.sync.dma_start(out=wt_sb[:], in_=WT[:])
            dinvD_sb = cpool.tile([P, prm.NSW * prm.TPSW], f32, tag="dd")
            nc.sync.dma_start(out=dinvD_sb[:], in_=dinvD[:])

            # ------- Phase A (replicated): table rows, bucket order -------
            with (
                tc.tile_pool(name="pa", bufs=4) as pa,
                tc.tile_pool(name="psa", bufs=4, space="PSUM") as psa,
            ):
                for g in range(prm.NG):
                    xt = pa.tile([P, prm.WG], bf16, tag="xt")
                    nc.sync.dma_start(
                        out=xt[:], in_=xT[:, g * prm.WG : (g + 1) * prm.WG]
                    )
                    hps = psa.tile([P, prm.J * FOUT], f32, tag="hps")
                    for j in range(prm.J):
                        nc.tensor.matmul(
                            out=hps[:, j * FOUT : (j + 1) * FOUT],
                            lhsT=xt[:, j * P : (j + 1) * P],
                            rhs=wt_sb[:],
                            start=True,
                            stop=True,
                        )
                    tsb = pa.tile([P, prm.J, 2 * FOUT], bf16, tag="tsb")
                    nc.vector.tensor_copy(
                        tsb[:, :, 0:FOUT],
                        hps[:].rearrange("p (j f) -> p j f", f=FOUT),
                    )
                    nc.vector.tensor_copy(
                        tsb[:, :, FOUT : 2 * FOUT],
                        hps[:].rearrange("p (j f) -> p j f", f=FOUT),
                    )
                    base = prm.WG * g
                    nc.sync.dma_start(
                        out=TBL[base : base + prm.WG, :].rearrange(
                            "(p j) f -> p j f", j=prm.J
                        ),
                        in_=tsb[:],
                    )

            # ------- Phase B: gather + one-hot matmuls -------
            qctr = [0]
            calls_by_sw = [[] for _ in range(prm.NSW)]
            for cm in calls:
                calls_by_sw[cm.sw].append(cm)
            S_MAX = max((cm.S for cm in calls), default=1)
            with (
                tc.tile_pool(name="pidx", bufs=6) as pidx,
                tc.tile_pool(name="pg", bufs=6) as pg,
                tc.tile_pool(name="pb", bufs=6) as pb,
                tc.tile_pool(name="py", bufs=2) as py,
                tc.tile_pool(name="psb", bufs=2, space="PSUM") as psb,
            ):
                for sw in range(prm.NSW):
                    if not calls_by_sw[sw]:
                        continue
                    tiles = {}
                    for cm in calls_by_sw[sw]:
                        S = cm.S
                        idx_t = pidx.tile([P, 8 * S_MAX], i16, tag="idx")
                        nc.scalar.dma_start(
                            out=idx_t[:, : 8 * S],
                            in_=gidx[:, cm.icol : cm.icol + 8 * S],
                        )
                        b_t = pb.tile([P, S_MAX, P], fp8, tag="b")
                        nc.sync.dma_start(
                            out=b_t[:, :S, :],
                            in_=bmat[:, cm.bcol * P : (cm.bcol + S) * P].rearrange(
                                "p (s q) -> p s q", q=P
                            ),
                        )
                        g_t = pg.tile([P, S_MAX, 2 * FOUT], bf16, tag="g")
                        nc.gpsimd.dma_gather(
                            out_ap=g_t[:, :S, :],
                            in_ap=TBL[
                                cm.bk * prm.BKCAP : min(
                                    (cm.bk + 1) * prm.BKCAP, prm.N2
                                ),
                                :,
                            ],
                            idxs_ap=idx_t[:, : 8 * S],
                            num_idxs=S * P,
                            num_idxs_reg=S * P,
                            elem_size=2 * FOUT,
                            single_packet=False,
                            queue_num=qctr[0] % 4,
                        )
                        qctr[0] += 1
                        tiles[(cm.bk, cm.k)] = (g_t, b_t)
                    psum_t = [
                        psb.tile([P, FOUT], f32, tag=f"acc{t}", name=f"acc{t}")
                        for t in range(prm.TPSW)
                    ]
                    for bk, s, t, st, sp in mms_by_sw[sw]:
                        g_t, b_t = tiles[(bk, s // prm.S_CAP)]
                        sl = s % prm.S_CAP
                        nc.tensor.matmul(
                            out=psum_t[t][:],
                            lhsT=b_t[:, sl, :],
                            rhs=g_t[:, sl, 0:FOUT],
                            start=st,
                            stop=sp,
                        )
                    rows_sw = min(prm.SWD, prm.NS - sw * prm.SWD)
                    nt = (rows_sw + P - 1) // P
                    ysb = py.tile([P, prm.TPSW, FOUT], f32, tag="ysb")
                    for t in range(nt):
                        w = sw * prm.TPSW + t
                        nc.scalar.activation(
                            out=ysb[:, t, :],
                            in_=psum_t[t][:],
                            func=mybir.ActivationFunctionType.Copy,
                            scale=dinvD_sb[:, w : w + 1],
                        )
                    for t in range(nt):
                        rt = min(P, rows_sw - t * P)
                        r0 = sw * prm.SWD + t * P
                        nc.scalar.dma_start(
                            out=y[r0 : r0 + rt, :], in_=ysb[:rt, t, :]
                        )

    nc.compile()
    _split_sync_waits(nc)
    return nc


def _get_program_and_prep(x, edge_index, W, prm):
    inputs, calls, mms_by_sw = _host_prep(x, edge_index, W, prm)
    icols = sum(8 * cm.S for cm in calls)
    bcols = sum(cm.S for cm in calls)
    nc = _build_program(prm, calls, mms_by_sw, icols, bcols)
    return nc, inputs


def kernel(x, edge_index, W):
    prm = Prm(N=int(x.shape[0]))
    nc, inputs = _get_program_and_prep(x, edge_index, W, prm)
    res = run_bass_kernel_spmd(nc, inputs, list(range(prm.C)))
    y = np.concatenate([res.results[c]["y"] for c in range(prm.C)], axis=0)
    return y.astype(np.float32)


def run_with_trace(x, edge_index, W, trace_cores=None):
    prm = Prm(N=int(x.shape[0]))
    nc, inputs = _get_program_and_prep(x, edge_index, W, prm)
    res = run_bass_kernel_spmd(
        nc, inputs, list(range(prm.C)), trace=True, trace_cores=trace_cores
    )
    y = np.concatenate([res.results[c]["y"] for c in range(prm.C)], axis=0)
    return y.astype(np.float32), res
